# revision 30
# baseline (speedup 1.0000x reference)
"""TRN2 Bass kernel for nn_ComboFwdVecComp (B=4, S=512, C=V=128).

out[b,i,j,v] = tanh( sum_c ctx[b,i,c] * ( Wm[v,c]*ctx[b,j,c] + (W2-Wd)[v,c] )
                     + A[b,j,v] + btot[v] )
  A = ctx @ (W1+Wd).T  (j-dep affine part),  btot = b1+b2+bm+bd.
The i-dep affine part (ctx_i @ (W2-Wd).T) folds into the main GEMM via the
(W2-Wd).T rhs term; the j-dep part becomes the K=1 bias matmul row.

fp16 version: output stored as fp16 (tanh is bounded; fp16 quantization
adds ~2.4e-4 abs err; host upcasts to f32), halving HBM store traffic vs
f32 (64 -> 32 MiB/core). PE runs fp16 (1 col/cycle warm vs f32r ~2x
slower). Measured rel err ~4e-3 vs the 2e-2 gate.

The pacer is the ACT engine: tanh [128,2048] f32->fp16 at ~1.9us per
half x 64 halves ~ 125us/core. Everything else is kept under that pace:
  - main mm rhs MUST be contiguous (j-major, v-inner pair buffer): a
    strided rhs AP ([1,4],[8,128]) measured 922ns per 512-col fp16 mm
    vs ~213ns contiguous -- the moving-operand SBUF feed collapses on
    16-byte-strided reads.
  - DVE prep: in (j,v) layout the mult's ctx_j operand would have a
    step-0 innermost dim (v-broadcast) -> 1x mode (~1.3us per [C,1024]
    op). Fix: ctx is DUPLICATED along j on the host (ctxT2[c,2j+r] =
    ctx[j,c]) so the operand presents innermost [1,2] step-1 pairs of
    equal fp16 values (4B-aligned) with the v-broadcast on a middle
    step-0 dim -- all operands then satisfy the 2x_1P packing rule
    (2-byte dtype, innermost step +-1). Both prep ops run 2x on DVE
    (~0.85us each per 8-j pair). A DVE+GPSIMD split was tried instead
    and REVERTED: Pool shares its SBUF port with DVE, and the measured
    contention ran both mult halves at ~1.4us (no faster than 1x DVE).
  - store DMAs rotate over the two HWDGE queues (SP/sync + ACT/scalar;
    vector has no DGE). HWDGE needs a 3D AP via max_dma_last_dim=1024
    to spread descriptors over the 16 SDMA engines.

Sharding: 8 cores, core k handles b = k//2, i in [ (k%2)*256, +256 ).
Each core emits out_shard (256, 512, 128) fp16 = 32 MiB; host
concatenates and upcasts.

Per-core structure: loop j-groups (32 j's = 4 pairs of 8), then i-chunks
(128 i's), then halves (4 psum banks = 16 j's):
  bias mm (K=1, N=512): ones^T @ browp_quad, strip-tiled on PE rows
      0/32/64/96 so the four bias mms run concurrently.
  main mm (K=128, N=512): ctxiT^T @ rhs'_quad accumulates on top, one
      LDW per half.
  ACT tanh drains the half [128,2048] f32 -> fp16 SBUF; ONE 0.5 MiB DMA.

A dummy tanh at build start preloads the ACT lookup table (otherwise the
first drain stalls ~9us mid-pipeline). Input DMA order per queue =
modeled completion order (Tile bakes it into semaphore waits).
"""

import sys
import types
from contextlib import ExitStack

import numpy as np

import concourse.bass as bass
import concourse.mybir as mybir
import concourse.tile as tile
from concourse import bacc
from concourse.bass_utils import run_bass_kernel_spmd

B, S, C, V = 4, 512, 128, 128
NCORES = 8
NI = 256          # i's per core
NQJ = S // 4      # j quads (128)
NJG = NQJ // 8    # j groups of 8 quads / 32 j's (16)

_F32 = mybir.dt.float32
_F16 = mybir.dt.float16


def install_ntff_shim():
    """antenv.axon_hooks is absent on some images; shim it so trace=True works."""
    if "antenv.axon_hooks" in sys.modules:
        return
    try:
        from trn_agent_boot.trn_boot import _ntff_profile_via_ctypes
        hook = _ntff_profile_via_ctypes("/opt/axon/libaxon_pjrt.so")
    except Exception:
        hook = None
    mod = types.ModuleType("antenv.axon_hooks")
    mod.get_axon_ntff_profile_hook = lambda: hook
    mod.set_axon_ntff_profile_hook = lambda h: None
    sys.modules["antenv.axon_hooks"] = mod


def build_nc():
    nc = bacc.Bacc("TRN2", target_bir_lowering=False, debug=False)

    BPW = (NQJ // 4) * 512  # browp row width (16384)

    # packed = [wmT (V) | w2mdT (V) | ctxT2 (2S)] so the ramp-critical
    # inputs arrive in ONE dma_start (each HWDGE config costs ~0.7us of
    # serial sequencer time; 7 separate input configs measured ~6us of
    # ramp). ctxT2 holds ctx DUPLICATED along j (ctxT2[c, 2j+r] =
    # ctx[j, c]): the prep mult's ctx_j operand then has an innermost
    # [1,2] step-1 dim (adjacent equal fp16 values, 4B-aligned pairs),
    # which satisfies the DVE 2x_1P packing rule; a plain v-broadcast
    # (innermost step 0) would force 1x mode (~2x slower).
    PKW = 2 * V + 2 * S
    packed_d = nc.dram_tensor("packed", [C, PKW], _F16, kind="ExternalInput").ap()
    ctxiT_d = nc.dram_tensor("ctxiT", [C, NI], _F16, kind="ExternalInput").ap()
    browp_d = nc.dram_tensor("browp", [4, BPW], _F16, kind="ExternalInput").ap()
    out_d = nc.dram_tensor("out_shard", [NI, S, V], _F16, kind="ExternalOutput").ap()

    with tile.TileContext(nc) as tc, ExitStack() as ctx:
        singles = ctx.enter_context(tc.tile_pool(name="singles", bufs=1))
        rhs_pool = ctx.enter_context(tc.tile_pool(name="rhs", bufs=8))
        tmp_pool = ctx.enter_context(tc.tile_pool(name="tmp", bufs=3))
        psum_pool = ctx.enter_context(tc.tile_pool(name="psum", bufs=1, space="PSUM"))
        out_pool = ctx.enter_context(tc.tile_pool(name="outs", bufs=8))

        # ---- load constants. Queue order = modeled completion order (the
        # Tile scheduler bakes it into semaphore waits): browp rows first
        # (the first bias mms gate on them), then the rhs'-prep deps
        # (wmq/w2mdrep/ctxT cols 0:32) and ctxiT, then the ctxT bulk. ----
        packed_sb = singles.tile([C, PKW], _F16)
        browp_r = singles.tile([97, BPW], _F16)
        wmT_sb = packed_sb[:, 0:V]
        w2mdT_sb = packed_sb[:, V:2 * V]
        ctxiT_r = singles.tile([C, NI], _F16)
        CT0 = 2 * V  # ctxT2 column offset inside packed

        def browp_ap(c0, c1, dram):
            # browp rows 0..3 live on partitions 0/32/64/96: one DMA with a
            # partition-step-32 AP instead of four per-row configs.
            if dram:
                return bass.AP(
                    tensor=browp_d.tensor, offset=browp_d.offset + c0,
                    ap=[[BPW, 4], [1, c1 - c0]],
                )
            return bass.AP(
                tensor=browp_r.tensor, offset=browp_r.offset + c0,
                ap=[[32 * browp_r.ap[0][0], 4], [1, c1 - c0]],
            )

        # ramp-critical first: the packed head (weights + ctxiT + first 2
        # pairs of ctxT2) on scalar, browp head (first j-group's quads) on
        # sync; bulk/tails after.
        nc.scalar.dma_start(
            out=packed_sb[:, 0:CT0 + 64], in_=packed_d[:, 0:CT0 + 64]
        )
        nc.sync.dma_start(out=browp_ap(0, 1024, False), in_=browp_ap(0, 1024, True))
        nc.sync.dma_start(out=ctxiT_r, in_=ctxiT_d)
        nc.scalar.dma_start(
            out=packed_sb[:, CT0 + 64:], in_=packed_d[:, CT0 + 64:]
        )
        nc.sync.dma_start(out=browp_ap(1024, BPW, False), in_=browp_ap(1024, BPW, True))

        ones_r = singles.tile([97, 128], _F16)
        nc.vector.memset(ones_r, 1.0)
        # Dummy activation: the ACT engine loads its tanh lookup table on
        # first use (~9us stall observed mid-pipeline); trigger the load now
        # so it overlaps the input DMAs instead of stalling the first drain.
        warm = singles.tile([97, 8], _F32)
        nc.scalar.activation(
            warm, ones_r[:, 0:8], mybir.ActivationFunctionType.Tanh
        )

        # one 8-bank psum megatile; bank b occupies [:, b*512:(b+1)*512]
        P = psum_pool.tile([128, 4096], _F32, name="mega")

        # SP and ACT are the HWDGE queues; Pool's SWDGE takes every third
        # store (its shredding costs ~1us of Pool time, which is free now).
        # 2 queues alone measured ~120 GB/s each -- right at the per-queue
        # chain-dispatch ceiling, backing stores up ~11us at the tail.
        dma_engines = [nc.sync, nc.gpsimd, nc.scalar]
        dma_i = 0

        def wv_bc(t):
            # weight [C, V] broadcast over 8 j's, iterated (jl, vhi, vlo):
            # jl step 0, v split [2,64],[1,2] so the innermost dim is
            # step-1 count-2 (2x_1P eligible).
            return bass.AP(
                tensor=t.tensor, offset=t.offset,
                ap=[t.ap[0], [0, 8], [2, V // 2], [1, 2]],
            )

        def prep_pair(gp):
            # rhs' for j's [8*gp, 8*gp+8) in (j-major, v-inner) layout:
            # rhs[c, jl*V+v] = wm[c,v]*ctx[8gp+jl, c] + w2md[c,v].
            # All operands present innermost step-1 2-byte dims (the ctx_j
            # operand via the duplicated ctxT2, [1,2] pairs; jl step 2) ->
            # both DVE ops run 2x_1P.
            tmp_p = tmp_pool.tile([C, 8 * V], _F16)
            ctxj2 = bass.AP(
                tensor=packed_sb.tensor,
                offset=packed_sb.offset + CT0 + 16 * gp,
                ap=[packed_sb.ap[0], [2, 8], [0, V // 2], [1, 2]],
            )
            nc.vector.tensor_tensor(
                out=tmp_p, in0=wv_bc(wmT_sb), in1=ctxj2, op=mybir.AluOpType.mult
            )
            rhs_p = rhs_pool.tile([C, 8 * V], _F16)
            nc.vector.tensor_tensor(
                out=rhs_p, in0=tmp_p, in1=wv_bc(w2mdT_sb), op=mybir.AluOpType.add
            )
            return rhs_p

        def pair_slice(pairs, qq):
            return pairs[qq // 2][:, (qq % 2) * 4 * V:(qq % 2 + 1) * 4 * V]

        for jg in range(NJG):
            if jg == 0:
                # ramp: only the first half's quads before the first matmuls
                pairs = [prep_pair(0), prep_pair(1), None, None]
            else:
                pairs = [prep_pair(4 * jg + pp) for pp in range(4)]

            for ic in range(2):
                for half in range(2):
                    first = jg == 0 and ic == 0 and half == 0
                    if first:
                        # ---- ramp special case: drain the first half as
                        # two [128,1024] bank-pairs so the first tanh only
                        # waits for pair 0's prep + matmuls (~1.7us earlier
                        # than waiting for pair 1 too). ----
                        for pp in range(2):
                            for s in (2 * pp, 2 * pp + 1):
                                q = 8 * jg + 4 * half + s
                                strip = (q % 4) * 32
                                col = (q // 4) * 512
                                nc.tensor.matmul(
                                    P[:, s * 512:(s + 1) * 512],
                                    lhsT=ones_r[strip:strip + 1, :],
                                    rhs=browp_r[strip:strip + 1, col:col + 512],
                                    start=True,
                                    stop=False,
                                    tile_position=(strip, 0),
                                )
                            for s in (2 * pp, 2 * pp + 1):
                                nc.tensor.matmul(
                                    P[:, s * 512:(s + 1) * 512],
                                    lhsT=ctxiT_r[:, ic * 128:(ic + 1) * 128],
                                    rhs=pair_slice(pairs, s),
                                    start=False,
                                    stop=True,
                                )
                            ot = out_pool.tile([128, 1024], _F16)
                            nc.scalar.activation(
                                ot, P[:, pp * 1024:(pp + 1) * 1024],
                                mybir.ActivationFunctionType.Tanh,
                            )
                            dst = bass.AP(
                                tensor=out_d.tensor,
                                offset=(pp * 8) * V,
                                ap=[[S * V, 128], [1, 8 * V]],
                            )
                            eng = dma_engines[dma_i % 3]
                            dma_i += 1
                            if eng is nc.gpsimd:
                                eng.dma_start(out=dst, in_=ot[:, :])
                            else:
                                eng.dma_start(
                                    out=dst, in_=ot[:, :], max_dma_last_dim=1024
                                )
                        pairs[2] = prep_pair(2)
                        pairs[3] = prep_pair(3)
                        continue
                    # ---- bias mms: 4 quads, strip-concurrent ----
                    for s in range(4):
                        q = 8 * jg + 4 * half + s
                        strip = (q % 4) * 32
                        col = (q // 4) * 512
                        bank = 4 * half + s
                        nc.tensor.matmul(
                            P[:, bank * 512:(bank + 1) * 512],
                            lhsT=ones_r[strip:strip + 1, :],
                            rhs=browp_r[strip:strip + 1, col:col + 512],
                            start=True,
                            stop=False,
                            tile_position=(strip, 0),
                        )
                    # ---- main mms: one ctxiT LDW per half ----
                    for s in range(4):
                        bank = 4 * half + s
                        nc.tensor.matmul(
                            P[:, bank * 512:(bank + 1) * 512],
                            lhsT=ctxiT_r[:, ic * 128:(ic + 1) * 128],
                            rhs=pair_slice(pairs, 4 * half + s),
                            start=False,
                            stop=True,
                        )

                    # ---- drain the half: tanh [128,2048] f32->fp16 +
                    # ONE 0.5 MiB store. HWDGE queues need a 3D AP (outer=
                    # 128 partitions) to spread descriptors across the 16
                    # SDMA engines -- a 2D row-list pins the whole chain on
                    # one engine; max_dma_last_dim=1024 -> [[1024,2],
                    # [1,1024]]: 2 KiB descriptors. SWDGE (gpsimd) shreds
                    # any shape itself. (Batching 2 halves per store was
                    # tried and reverted: the fixed ~8us teardown does not
                    # scale with store count, and the bigger final store
                    # lengthens the tail.)
                    ot = out_pool.tile([128, 2048], _F16)
                    nc.scalar.activation(
                        ot, P[:, half * 2048:(half + 1) * 2048],
                        mybir.ActivationFunctionType.Tanh,
                    )
                    dst = bass.AP(
                        tensor=out_d.tensor,
                        offset=(ic * 128) * S * V + (jg * 32 + half * 16) * V,
                        ap=[[S * V, 128], [1, 16 * V]],
                    )
                    eng = dma_engines[dma_i % 3]
                    dma_i += 1
                    if eng is nc.gpsimd:
                        eng.dma_start(out=dst, in_=ot[:, :])
                    else:
                        eng.dma_start(out=dst, in_=ot[:, :], max_dma_last_dim=1024)

    nc.compile()
    return nc


_NC_CACHE = {}


def get_nc():
    if "nc" not in _NC_CACHE:
        _NC_CACHE["nc"] = build_nc()
    return _NC_CACHE["nc"]


def make_in_maps(ctx, W1, b1, W2, b2, Wm, bm, Wd, bd):
    ctx = np.asarray(ctx, np.float32)
    btot = (
        np.asarray(b1) + np.asarray(b2) + np.asarray(bm) + np.asarray(bd)
    ).astype(np.float32)
    wmT = np.ascontiguousarray(np.asarray(Wm, np.float32).T)                  # (C,V)
    w2mdT = np.ascontiguousarray(
        (np.asarray(W2) - np.asarray(Wd)).T.astype(np.float32)
    )
    w1d = (np.asarray(W1) + np.asarray(Wd)).astype(np.float32)                # (V,C)

    wmTh = wmT.astype(np.float16)                                             # (C,V)
    w2mdTh = w2mdT.astype(np.float16)                                         # (C,V)

    per_b = []
    for b in range(B):
        A = (ctx[b] @ w1d.T + btot).astype(np.float32)                        # (S,V)
        browq = A.reshape(NQJ, 4 * V)                                         # quad rows
        browp = np.zeros((4, (NQJ // 4) * 512), np.float16)
        for q in range(NQJ):
            browp[q % 4, (q // 4) * 512:(q // 4) * 512 + 512] = browq[q]
        ctxT2 = np.repeat(ctx[b].T.astype(np.float16), 2, axis=1)             # (C,2S)
        packed = np.ascontiguousarray(
            np.concatenate([wmTh, w2mdTh, ctxT2], axis=1)                     # (C,PKW)
        )
        per_b.append((packed, browp))

    in_maps = []
    for k in range(NCORES):
        b = k // 2
        i0c = (k % 2) * NI
        packed, browp = per_b[b]
        in_maps.append({
            "packed": packed,
            "ctxiT": np.ascontiguousarray(ctx[b, i0c:i0c + NI].T.astype(np.float16)),
            "browp": browp,
        })
    return in_maps


def run(in_maps, **kw):
    return run_bass_kernel_spmd(get_nc(), in_maps, core_ids=list(range(NCORES)), **kw)


def assemble(results):
    out = np.empty((B, S, S, V), np.float32)
    for k in range(NCORES):
        b = k // 2
        i0c = (k % 2) * NI
        out[b, i0c:i0c + NI] = np.asarray(results[k]["out_shard"], np.float32)
    return out


def kernel(ctx, W1, b1, W2, b2, Wm, bm, Wd, bd):
    install_ntff_shim()
    in_maps = make_in_maps(ctx, W1, b1, W2, b2, Wm, bm, Wd, bd)
    res = run(in_maps)
    return assemble(res.results)


# revision 31
# speedup vs baseline: 1.0023x; 1.0023x over previous
"""TRN2 Bass kernel for nn_ComboFwdVecComp (B=4, S=512, C=V=128).

out[b,i,j,v] = tanh( sum_c ctx[b,i,c] * ( Wm[v,c]*ctx[b,j,c] + (W2-Wd)[v,c] )
                     + A[b,j,v] + btot[v] )
  A = ctx @ (W1+Wd).T  (j-dep affine part),  btot = b1+b2+bm+bd.
The i-dep affine part (ctx_i @ (W2-Wd).T) folds into the main GEMM via the
(W2-Wd).T rhs term; the j-dep part becomes the K=1 bias matmul row.

fp16 version: output stored as fp16 (tanh is bounded; fp16 quantization
adds ~2.4e-4 abs err; host upcasts to f32), halving HBM store traffic vs
f32 (64 -> 32 MiB/core). PE runs fp16 (1 col/cycle warm vs f32r ~2x
slower). Measured rel err ~4e-3 vs the 2e-2 gate.

The pacer is the ACT engine: tanh [128,2048] f32->fp16 at ~1.9us per
half x 64 halves ~ 125us/core. Everything else is kept under that pace:
  - main mm rhs MUST be contiguous (j-major, v-inner pair buffer): a
    strided rhs AP ([1,4],[8,128]) measured 922ns per 512-col fp16 mm
    vs ~213ns contiguous -- the moving-operand SBUF feed collapses on
    16-byte-strided reads.
  - DVE prep: in (j,v) layout the mult's ctx_j operand would have a
    step-0 innermost dim (v-broadcast) -> 1x mode (~1.3us per [C,1024]
    op). Fix: ctx is DUPLICATED along j on the host (ctxT2[c,2j+r] =
    ctx[j,c]) so the operand presents innermost [1,2] step-1 pairs of
    equal fp16 values (4B-aligned) with the v-broadcast on a middle
    step-0 dim -- all operands then satisfy the 2x_1P packing rule
    (2-byte dtype, innermost step +-1). Both prep ops run 2x on DVE
    (~0.85us each per 8-j pair). A DVE+GPSIMD split was tried instead
    and REVERTED: Pool shares its SBUF port with DVE, and the measured
    contention ran both mult halves at ~1.4us (no faster than 1x DVE).
  - store DMAs rotate over three queues: the two HWDGE queues (SP/sync
    + ACT/scalar; vector has no DGE) and Pool's SWDGE. HWDGE needs a 3D
    AP via max_dma_last_dim=1024 to spread descriptors over the 16 SDMA
    engines. Two queues alone measured ~120 GB/s each (the per-queue
    chain-dispatch ceiling) and backed stores up ~11us at the tail.
  - ramp: all ramp-critical inputs arrive in one packed dma_start per
    queue (each HWDGE config costs ~0.7us of serial sequencer time),
    browp's 4 rows load via a single partition-step-32 AP, and the
    first half drains as two [128,1024] bank-pairs so the first tanh
    only waits for pair 0's prep. ctxiT must stay a dedicated tile:
    LDWEIGHTS from a column-offset slice of a shared tile fails at
    runtime (INVALID_ARGUMENT).

Measured: ~140us (vs 204.5us f32r baseline). ACT busy ~120us = 86% of
exec; ~6us ramp + ~8us fixed Tile teardown (semaphore drain) remain.
Run-to-run clock throttling (1.2 -> 1.0 GHz) adds up to +20%.

Sharding: 8 cores, core k handles b = k//2, i in [ (k%2)*256, +256 ).
Each core emits out_shard (256, 512, 128) fp16 = 32 MiB; host
concatenates and upcasts.

Per-core structure: loop j-groups (32 j's = 4 pairs of 8), then i-chunks
(128 i's), then halves (4 psum banks = 16 j's):
  bias mm (K=1, N=512): ones^T @ browp_quad, strip-tiled on PE rows
      0/32/64/96 so the four bias mms run concurrently.
  main mm (K=128, N=512): ctxiT^T @ rhs'_quad accumulates on top, one
      LDW per half.
  ACT tanh drains the half [128,2048] f32 -> fp16 SBUF; ONE 0.5 MiB DMA.

A dummy tanh at build start preloads the ACT lookup table (otherwise the
first drain stalls ~9us mid-pipeline). Input DMA order per queue =
modeled completion order (Tile bakes it into semaphore waits).
"""

import sys
import types
from contextlib import ExitStack

import numpy as np

import concourse.bass as bass
import concourse.mybir as mybir
import concourse.tile as tile
from concourse import bacc
from concourse.bass_utils import run_bass_kernel_spmd

B, S, C, V = 4, 512, 128, 128
NCORES = 8
NI = 256          # i's per core
NQJ = S // 4      # j quads (128)
NJG = NQJ // 8    # j groups of 8 quads / 32 j's (16)

_F32 = mybir.dt.float32
_F16 = mybir.dt.float16


def install_ntff_shim():
    """antenv.axon_hooks is absent on some images; shim it so trace=True works."""
    if "antenv.axon_hooks" in sys.modules:
        return
    try:
        from trn_agent_boot.trn_boot import _ntff_profile_via_ctypes
        hook = _ntff_profile_via_ctypes("/opt/axon/libaxon_pjrt.so")
    except Exception:
        hook = None
    mod = types.ModuleType("antenv.axon_hooks")
    mod.get_axon_ntff_profile_hook = lambda: hook
    mod.set_axon_ntff_profile_hook = lambda h: None
    sys.modules["antenv.axon_hooks"] = mod


def build_nc():
    nc = bacc.Bacc("TRN2", target_bir_lowering=False, debug=False)

    BPW = (NQJ // 4) * 512  # browp row width (16384)

    # packed = [wmT (V) | w2mdT (V) | ctxT2 (2S)] so the ramp-critical
    # inputs arrive in ONE dma_start (each HWDGE config costs ~0.7us of
    # serial sequencer time; 7 separate input configs measured ~6us of
    # ramp). ctxT2 holds ctx DUPLICATED along j (ctxT2[c, 2j+r] =
    # ctx[j, c]): the prep mult's ctx_j operand then has an innermost
    # [1,2] step-1 dim (adjacent equal fp16 values, 4B-aligned pairs),
    # which satisfies the DVE 2x_1P packing rule; a plain v-broadcast
    # (innermost step 0) would force 1x mode (~2x slower).
    PKW = 2 * V + 2 * S
    packed_d = nc.dram_tensor("packed", [C, PKW], _F16, kind="ExternalInput").ap()
    ctxiT_d = nc.dram_tensor("ctxiT", [C, NI], _F16, kind="ExternalInput").ap()
    browp_d = nc.dram_tensor("browp", [4, BPW], _F16, kind="ExternalInput").ap()
    out_d = nc.dram_tensor("out_shard", [NI, S, V], _F16, kind="ExternalOutput").ap()

    with tile.TileContext(nc) as tc, ExitStack() as ctx:
        singles = ctx.enter_context(tc.tile_pool(name="singles", bufs=1))
        rhs_pool = ctx.enter_context(tc.tile_pool(name="rhs", bufs=8))
        tmp_pool = ctx.enter_context(tc.tile_pool(name="tmp", bufs=3))
        psum_pool = ctx.enter_context(tc.tile_pool(name="psum", bufs=1, space="PSUM"))
        out_pool = ctx.enter_context(tc.tile_pool(name="outs", bufs=8))

        # ---- load constants. Queue order = modeled completion order (the
        # Tile scheduler bakes it into semaphore waits): browp rows first
        # (the first bias mms gate on them), then the rhs'-prep deps
        # (wmq/w2mdrep/ctxT cols 0:32) and ctxiT, then the ctxT bulk. ----
        packed_sb = singles.tile([C, PKW], _F16)
        browp_r = singles.tile([97, BPW], _F16)
        wmT_sb = packed_sb[:, 0:V]
        w2mdT_sb = packed_sb[:, V:2 * V]
        ctxiT_r = singles.tile([C, NI], _F16)
        CT0 = 2 * V  # ctxT2 column offset inside packed

        def browp_ap(c0, c1, dram):
            # browp rows 0..3 live on partitions 0/32/64/96: one DMA with a
            # partition-step-32 AP instead of four per-row configs.
            if dram:
                return bass.AP(
                    tensor=browp_d.tensor, offset=browp_d.offset + c0,
                    ap=[[BPW, 4], [1, c1 - c0]],
                )
            return bass.AP(
                tensor=browp_r.tensor, offset=browp_r.offset + c0,
                ap=[[32 * browp_r.ap[0][0], 4], [1, c1 - c0]],
            )

        # ramp-critical first: the packed head (weights + ctxiT + first 2
        # pairs of ctxT2) on scalar, browp head (first j-group's quads) on
        # sync; bulk/tails after.
        nc.scalar.dma_start(
            out=packed_sb[:, 0:CT0 + 64], in_=packed_d[:, 0:CT0 + 64]
        )
        nc.sync.dma_start(out=browp_ap(0, 1024, False), in_=browp_ap(0, 1024, True))
        nc.sync.dma_start(out=ctxiT_r, in_=ctxiT_d)
        nc.scalar.dma_start(
            out=packed_sb[:, CT0 + 64:], in_=packed_d[:, CT0 + 64:]
        )
        nc.sync.dma_start(out=browp_ap(1024, BPW, False), in_=browp_ap(1024, BPW, True))

        ones_r = singles.tile([97, 128], _F16)
        nc.vector.memset(ones_r, 1.0)
        # Dummy activation: the ACT engine loads its tanh lookup table on
        # first use (~9us stall observed mid-pipeline); trigger the load now
        # so it overlaps the input DMAs instead of stalling the first drain.
        warm = singles.tile([97, 8], _F32)
        nc.scalar.activation(
            warm, ones_r[:, 0:8], mybir.ActivationFunctionType.Tanh
        )

        # one 8-bank psum megatile; bank b occupies [:, b*512:(b+1)*512]
        P = psum_pool.tile([128, 4096], _F32, name="mega")

        # SP and ACT are the HWDGE queues; Pool's SWDGE takes every third
        # store (its shredding costs ~1us of Pool time, which is free now).
        # 2 queues alone measured ~120 GB/s each -- right at the per-queue
        # chain-dispatch ceiling, backing stores up ~11us at the tail.
        dma_engines = [nc.sync, nc.gpsimd, nc.scalar]
        dma_i = 0

        def wv_bc(t):
            # weight [C, V] broadcast over 8 j's, iterated (jl, vhi, vlo):
            # jl step 0, v split [2,64],[1,2] so the innermost dim is
            # step-1 count-2 (2x_1P eligible).
            return bass.AP(
                tensor=t.tensor, offset=t.offset,
                ap=[t.ap[0], [0, 8], [2, V // 2], [1, 2]],
            )

        def prep_pair(gp):
            # rhs' for j's [8*gp, 8*gp+8) in (j-major, v-inner) layout:
            # rhs[c, jl*V+v] = wm[c,v]*ctx[8gp+jl, c] + w2md[c,v].
            # All operands present innermost step-1 2-byte dims (the ctx_j
            # operand via the duplicated ctxT2, [1,2] pairs; jl step 2) ->
            # both DVE ops run 2x_1P.
            tmp_p = tmp_pool.tile([C, 8 * V], _F16)
            ctxj2 = bass.AP(
                tensor=packed_sb.tensor,
                offset=packed_sb.offset + CT0 + 16 * gp,
                ap=[packed_sb.ap[0], [2, 8], [0, V // 2], [1, 2]],
            )
            nc.vector.tensor_tensor(
                out=tmp_p, in0=wv_bc(wmT_sb), in1=ctxj2, op=mybir.AluOpType.mult
            )
            rhs_p = rhs_pool.tile([C, 8 * V], _F16)
            nc.vector.tensor_tensor(
                out=rhs_p, in0=tmp_p, in1=wv_bc(w2mdT_sb), op=mybir.AluOpType.add
            )
            return rhs_p

        def pair_slice(pairs, qq):
            return pairs[qq // 2][:, (qq % 2) * 4 * V:(qq % 2 + 1) * 4 * V]

        for jg in range(NJG):
            if jg == 0:
                # ramp: only the first half's quads before the first matmuls
                pairs = [prep_pair(0), prep_pair(1), None, None]
            else:
                pairs = [prep_pair(4 * jg + pp) for pp in range(4)]

            for ic in range(2):
                for half in range(2):
                    first = jg == 0 and ic == 0 and half == 0
                    if first:
                        # ---- ramp special case: drain the first half as
                        # two [128,1024] bank-pairs so the first tanh only
                        # waits for pair 0's prep + matmuls (~1.7us earlier
                        # than waiting for pair 1 too). ----
                        for pp in range(2):
                            for s in (2 * pp, 2 * pp + 1):
                                q = 8 * jg + 4 * half + s
                                strip = (q % 4) * 32
                                col = (q // 4) * 512
                                nc.tensor.matmul(
                                    P[:, s * 512:(s + 1) * 512],
                                    lhsT=ones_r[strip:strip + 1, :],
                                    rhs=browp_r[strip:strip + 1, col:col + 512],
                                    start=True,
                                    stop=False,
                                    tile_position=(strip, 0),
                                )
                            for s in (2 * pp, 2 * pp + 1):
                                nc.tensor.matmul(
                                    P[:, s * 512:(s + 1) * 512],
                                    lhsT=ctxiT_r[:, ic * 128:(ic + 1) * 128],
                                    rhs=pair_slice(pairs, s),
                                    start=False,
                                    stop=True,
                                )
                            ot = out_pool.tile([128, 1024], _F16)
                            nc.scalar.activation(
                                ot, P[:, pp * 1024:(pp + 1) * 1024],
                                mybir.ActivationFunctionType.Tanh,
                            )
                            dst = bass.AP(
                                tensor=out_d.tensor,
                                offset=(pp * 8) * V,
                                ap=[[S * V, 128], [1, 8 * V]],
                            )
                            eng = dma_engines[dma_i % 3]
                            dma_i += 1
                            if eng is nc.gpsimd:
                                eng.dma_start(out=dst, in_=ot[:, :])
                            else:
                                eng.dma_start(
                                    out=dst, in_=ot[:, :], max_dma_last_dim=1024
                                )
                        pairs[2] = prep_pair(2)
                        pairs[3] = prep_pair(3)
                        continue
                    # ---- bias mms: 4 quads, strip-concurrent ----
                    for s in range(4):
                        q = 8 * jg + 4 * half + s
                        strip = (q % 4) * 32
                        col = (q // 4) * 512
                        bank = 4 * half + s
                        nc.tensor.matmul(
                            P[:, bank * 512:(bank + 1) * 512],
                            lhsT=ones_r[strip:strip + 1, :],
                            rhs=browp_r[strip:strip + 1, col:col + 512],
                            start=True,
                            stop=False,
                            tile_position=(strip, 0),
                        )
                    # ---- main mms: one ctxiT LDW per half ----
                    for s in range(4):
                        bank = 4 * half + s
                        nc.tensor.matmul(
                            P[:, bank * 512:(bank + 1) * 512],
                            lhsT=ctxiT_r[:, ic * 128:(ic + 1) * 128],
                            rhs=pair_slice(pairs, 4 * half + s),
                            start=False,
                            stop=True,
                        )

                    # ---- drain the half: tanh [128,2048] f32->fp16 +
                    # ONE 0.5 MiB store. HWDGE queues need a 3D AP (outer=
                    # 128 partitions) to spread descriptors across the 16
                    # SDMA engines -- a 2D row-list pins the whole chain on
                    # one engine; max_dma_last_dim=1024 -> [[1024,2],
                    # [1,1024]]: 2 KiB descriptors. SWDGE (gpsimd) shreds
                    # any shape itself. (Batching 2 halves per store was
                    # tried and reverted: the fixed ~8us teardown does not
                    # scale with store count, and the bigger final store
                    # lengthens the tail.)
                    ot = out_pool.tile([128, 2048], _F16)
                    nc.scalar.activation(
                        ot, P[:, half * 2048:(half + 1) * 2048],
                        mybir.ActivationFunctionType.Tanh,
                    )
                    dst = bass.AP(
                        tensor=out_d.tensor,
                        offset=(ic * 128) * S * V + (jg * 32 + half * 16) * V,
                        ap=[[S * V, 128], [1, 16 * V]],
                    )
                    eng = dma_engines[dma_i % 3]
                    dma_i += 1
                    if eng is nc.gpsimd:
                        eng.dma_start(out=dst, in_=ot[:, :])
                    else:
                        eng.dma_start(out=dst, in_=ot[:, :], max_dma_last_dim=1024)

    nc.compile()
    return nc


_NC_CACHE = {}


def get_nc():
    if "nc" not in _NC_CACHE:
        _NC_CACHE["nc"] = build_nc()
    return _NC_CACHE["nc"]


def make_in_maps(ctx, W1, b1, W2, b2, Wm, bm, Wd, bd):
    ctx = np.asarray(ctx, np.float32)
    btot = (
        np.asarray(b1) + np.asarray(b2) + np.asarray(bm) + np.asarray(bd)
    ).astype(np.float32)
    wmT = np.ascontiguousarray(np.asarray(Wm, np.float32).T)                  # (C,V)
    w2mdT = np.ascontiguousarray(
        (np.asarray(W2) - np.asarray(Wd)).T.astype(np.float32)
    )
    w1d = (np.asarray(W1) + np.asarray(Wd)).astype(np.float32)                # (V,C)

    wmTh = wmT.astype(np.float16)                                             # (C,V)
    w2mdTh = w2mdT.astype(np.float16)                                         # (C,V)

    per_b = []
    for b in range(B):
        A = (ctx[b] @ w1d.T + btot).astype(np.float32)                        # (S,V)
        browq = A.reshape(NQJ, 4 * V)                                         # quad rows
        browp = np.zeros((4, (NQJ // 4) * 512), np.float16)
        for q in range(NQJ):
            browp[q % 4, (q // 4) * 512:(q // 4) * 512 + 512] = browq[q]
        ctxT2 = np.repeat(ctx[b].T.astype(np.float16), 2, axis=1)             # (C,2S)
        packed = np.ascontiguousarray(
            np.concatenate([wmTh, w2mdTh, ctxT2], axis=1)                     # (C,PKW)
        )
        per_b.append((packed, browp))

    in_maps = []
    for k in range(NCORES):
        b = k // 2
        i0c = (k % 2) * NI
        packed, browp = per_b[b]
        in_maps.append({
            "packed": packed,
            "ctxiT": np.ascontiguousarray(ctx[b, i0c:i0c + NI].T.astype(np.float16)),
            "browp": browp,
        })
    return in_maps


def run(in_maps, **kw):
    return run_bass_kernel_spmd(get_nc(), in_maps, core_ids=list(range(NCORES)), **kw)


def assemble(results):
    out = np.empty((B, S, S, V), np.float32)
    for k in range(NCORES):
        b = k // 2
        i0c = (k % 2) * NI
        out[b, i0c:i0c + NI] = np.asarray(results[k]["out_shard"], np.float32)
    return out


def kernel(ctx, W1, b1, W2, b2, Wm, bm, Wd, bd):
    install_ntff_shim()
    in_maps = make_in_maps(ctx, W1, b1, W2, b2, Wm, bm, Wd, bd)
    res = run(in_maps)
    return assemble(res.results)
